# revision 9
# baseline (speedup 1.0000x reference)
"""Trainium2 Bass kernel for the roberta entity-span classification head.

Problem: nn_R_roberta_70360154243670 (segment_reduce, memory-bound).

  e1, e2 = per-example mean of last_hidden_states over the 1st / 2nd
           contiguous run of 1s in ent_ids
  p  = pooled @ W_cls + b_cls
  o1 = tanh(e1) @ W_e1 + b_e1 ; o2 = tanh(e2) @ W_e2 + b_e2
  logits = [p|o1|o2] @ W_cat + b_cat
  probs  = softmax(logits); loss = -mean(log_softmax(probs)[label])

Sharding: pure data parallel, batch 64 -> 8 cores x 8 examples,
small weights replicated to every core.

Device-side algorithm (per core, BL=8 examples):
  * span ids from ent_ids: starts = (ent[t] > ent[t-1]); inclusive cumsum
    over T via a triangular-ones matmul in [T-partition, batch-free]
    layout; span = cumsum * ent; m1 = (span==1), m2 = (span==2).
  * masked segment sums: one [16,768] PSUM accumulation over 32 matmuls
    (8 examples x 4 T-chunks) with zero-padded block-column masks as
    lhsT (fp32r: 1 cycle/row) streaming each h tile straight from DMA.
  * counts via ones-matmul -> 1/max(c,1) -> tanh(e * recip) on ACT.
  * tail kept in transposed [feature-partition, batch-free] layout so
    the weight matrices are used as matmul lhsT in their natural DRAM
    layout; biases are added as rank-1 (K=1) matmuls.
  * softmax + log-softmax + label pick on device; host only concatenates
    per-core outputs and averages 64 scalars.
"""

import numpy as np

import concourse.bass as bass
import concourse.mybir as mybir
import concourse.tile as tile
from concourse import bacc
from concourse.masks import make_identity, make_upper_triangular

B, T, H, L = 64, 512, 768, 30
NCORES = 8
BL = B // NCORES          # 8 examples per core
TC = T // 128             # 4 T-chunks
HC = H // 128             # 6 H-chunks
KC = 3 * HC               # 18 k-chunks of the concat dim

F32 = mybir.dt.float32
F32R = mybir.dt.float32r
I32 = mybir.dt.int32
AF = mybir.ActivationFunctionType
OP = mybir.AluOpType

_CACHE = {}


def build_nc():
    nc = bacc.Bacc("TRN2", target_bir_lowering=False)

    hid_d = nc.dram_tensor("hid", [BL, T, H], F32, kind="ExternalInput")
    pooled_d = nc.dram_tensor("pooled", [BL, H], F32, kind="ExternalInput")
    ent_d = nc.dram_tensor("ent", [BL, T], I32, kind="ExternalInput")
    lab_d = nc.dram_tensor("labels", [BL, 1], I32, kind="ExternalInput")
    wcls_d = nc.dram_tensor("w_cls", [H, H], F32, kind="ExternalInput")
    we1_d = nc.dram_tensor("w_e1", [H, H], F32, kind="ExternalInput")
    we2_d = nc.dram_tensor("w_e2", [H, H], F32, kind="ExternalInput")
    bcls_d = nc.dram_tensor("b_cls", [1, H], F32, kind="ExternalInput")
    be1_d = nc.dram_tensor("b_e1", [1, H], F32, kind="ExternalInput")
    be2_d = nc.dram_tensor("b_e2", [1, H], F32, kind="ExternalInput")
    wcat_d = nc.dram_tensor("w_cat", [3 * H, L], F32, kind="ExternalInput")
    bcat_d = nc.dram_tensor("b_cat", [1, L], F32, kind="ExternalInput")
    zeros_d = nc.dram_tensor("zeros", [128, TC * BL * 2 * BL], F32,
                             kind="ExternalInput")
    out_d = nc.dram_tensor("out", [BL, L + 1], F32, kind="ExternalOutput")

    with tile.TileContext(nc) as tc:
        with (
            tc.tile_pool(name="const", bufs=1) as cpool,
            tc.tile_pool(name="wpool", bufs=1) as wpool,
            tc.tile_pool(name="hpool", bufs=3) as hpool,
            tc.tile_pool(name="spool", bufs=1) as spool,
            tc.tile_pool(name="ps2", bufs=3, space="PSUM") as ps2,
            tc.tile_pool(name="ps1", bufs=1, space="PSUM") as ps1,
            tc.tile_pool(name="pse", bufs=1, space="PSUM") as pse,
        ):
            # ---- constants built on device ----
            ones128 = cpool.tile([128, 128], F32)
            nc.vector.memset(ones128, 1.0)
            triu = cpool.tile([128, 128], F32)
            make_upper_triangular(nc, triu, val=1.0, diag=True)
            ident = cpool.tile([128, 128], F32)
            make_identity(nc, ident)
            iota30 = cpool.tile([BL, L], F32)
            nc.gpsimd.iota(iota30, pattern=[[1, L]], base=0,
                           channel_multiplier=0,
                           allow_small_or_imprecise_dtypes=True)

            # ---- small input DMAs ----
            ent_sb = spool.tile([BL, T], I32)
            nc.sync.dma_start(out=ent_sb, in_=ent_d[:])
            pooled_sb = spool.tile([BL, H], F32)
            nc.sync.dma_start(out=pooled_sb, in_=pooled_d[:])
            lab_sb = spool.tile([BL, 1], I32)
            nc.sync.dma_start(out=lab_sb, in_=lab_d[:])
            bcls_sb = spool.tile([1, H], F32)
            nc.sync.dma_start(out=bcls_sb, in_=bcls_d[:])
            be1_sb = spool.tile([1, H], F32)
            nc.sync.dma_start(out=be1_sb, in_=be1_d[:])
            be2_sb = spool.tile([1, H], F32)
            nc.sync.dma_start(out=be2_sb, in_=be2_d[:])
            bcat_sb = spool.tile([1, L], F32)
            nc.sync.dma_start(out=bcat_sb, in_=bcat_d[:])

            # ---- span-id mask pipeline (cheap, runs during first h DMA) ----
            entf = spool.tile([BL, T + 1], F32)
            nc.vector.memset(entf[:, 0:1], 0.0)
            nc.vector.tensor_copy(out=entf[:, 1:T + 1], in_=ent_sb)
            starts = spool.tile([BL, T], F32)
            nc.vector.tensor_tensor(out=starts, in0=entf[:, 1:T + 1],
                                    in1=entf[:, 0:T], op=OP.is_gt)

            startsT = spool.tile([128, TC, BL], F32)
            entT = spool.tile([128, TC, BL], F32)
            for c in range(TC):
                pt = ps2.tile([128, 16], F32, tag="ps2")
                nc.tensor.transpose(pt[:, 0:BL], starts[:, c * 128:(c + 1) * 128],
                                    ident[:BL, :BL])
                nc.vector.tensor_copy(out=startsT[:, c, :], in_=pt[:, 0:BL])
                pt2 = ps2.tile([128, 16], F32, tag="ps2")
                nc.tensor.transpose(pt2[:, 0:BL],
                                    entf[:, 1 + c * 128:1 + (c + 1) * 128],
                                    ident[:BL, :BL])
                nc.vector.tensor_copy(out=entT[:, c, :], in_=pt2[:, 0:BL])

            spanT = spool.tile([128, TC, BL], F32)
            for mc in range(TC):
                pc = ps2.tile([128, 16], F32, tag="ps2")
                for kc in range(mc + 1):
                    nc.tensor.matmul(pc[:, 0:BL],
                                     triu if kc == mc else ones128,
                                     startsT[:, kc, :],
                                     start=(kc == 0), stop=(kc == mc))
                nc.vector.tensor_mul(spanT[:, mc, :], pc[:, 0:BL], entT[:, mc, :])

            masks = spool.tile([128, TC, BL, 2], F32)
            for c in range(TC):
                nc.vector.tensor_scalar(out=masks[:, c, :, 0], in0=spanT[:, c, :],
                                        scalar1=1.0, scalar2=None, op0=OP.is_equal)
                nc.vector.tensor_scalar(out=masks[:, c, :, 1], in0=spanT[:, c, :],
                                        scalar1=2.0, scalar2=None, op0=OP.is_equal)

            # zero-padded block-column masks: col 2b+j of masksZ[:,c,b,:] holds
            # mask j of example b, other columns zero, so each (b,c) matmul
            # accumulates only into rows 2b:2b+2 of the shared [16,768] psum.
            masksZ = spool.tile([128, TC, BL, 2 * BL], F32R)
            nc.sync.dma_start(out=masksZ, in_=zeros_d[:].bitcast(F32R)
                              .rearrange("p (c b k) -> p c b k", c=TC, b=BL))
            for c in range(TC):
                for b in range(BL):
                    nc.vector.tensor_copy(out=masksZ[:, c, b, 2 * b:2 * b + 2],
                                          in_=masks[:, c, b, :])

            # counts -> 1/max(c,1), rows 2b+j
            pcnt = ps1.tile([16, 1], F32, tag="cnt")
            for c in range(TC):
                nc.tensor.matmul(pcnt, masks[:, c], ones128[:, 0:1],
                                 start=(c == 0), stop=(c == TC - 1))
            cnt_sb = spool.tile([16, 1], F32)
            nc.vector.tensor_scalar_max(cnt_sb, pcnt, 1.0)
            recip = spool.tile([16, 1], F32)
            nc.vector.reciprocal(recip, cnt_sb)

            # pooled transposed early (tail weight matmuls run mid-stream)
            pooledT = spool.tile([128, HC, BL], F32)
            for hc in range(HC):
                pp = ps2.tile([128, 16], F32, tag="ps2")
                nc.tensor.transpose(pp[:, 0:BL],
                                    pooled_sb[:, hc * 128:(hc + 1) * 128],
                                    ident[:BL, :BL])
                nc.vector.tensor_copy(out=pooledT[:, hc, :], in_=pp[:, 0:BL])

            # ---- weight DMAs (interleaved with h stream below) ----
            wcls_sb = wpool.tile([128, HC, H], F32)
            we1_sb = wpool.tile([128, HC, H], F32)
            we2_sb = wpool.tile([128, HC, H], F32)
            wcat_sb = wpool.tile([128, KC, L], F32)

            hv = hid_d[:].rearrange("b (c p) d -> p b c d", p=128)

            # ---- h stream + masked segment-sum matmuls (fp32r) ----
            pe1 = pse.tile([16, 512], F32, tag="pe1")
            pe2 = pse.tile([16, 256], F32, tag="pe2")
            for b in range(BL):
                ht = hpool.tile([128, TC, H], F32R, tag="h")
                nc.sync.dma_start(out=ht, in_=hv[:, b].bitcast(F32R))
                for c in range(TC):
                    lhsT = masksZ[:, c, b, :]
                    nc.tensor.matmul(pe1, lhsT, ht[:, c, 0:512],
                                     start=(b == 0 and c == 0),
                                     stop=(b == BL - 1 and c == TC - 1))
                    nc.tensor.matmul(pe2, lhsT, ht[:, c, 512:H],
                                     start=(b == 0 and c == 0),
                                     stop=(b == BL - 1 and c == TC - 1))
                # weight loads threaded between example streams so they are
                # on-chip before the tail but do not delay the first h tiles
                if b == 1:
                    nc.sync.dma_start(out=wcls_sb,
                                      in_=wcls_d[:].rearrange("(c p) m -> p c m", p=128))
                if b == 2:
                    nc.sync.dma_start(out=we1_sb,
                                      in_=we1_d[:].rearrange("(c p) m -> p c m", p=128))
                if b == 3:
                    nc.sync.dma_start(out=we2_sb,
                                      in_=we2_d[:].rearrange("(c p) m -> p c m", p=128))
                if b == 4:
                    nc.sync.dma_start(out=wcat_sb,
                                      in_=wcat_d[:].rearrange("(c p) l -> p c l", p=128))

            # concatT slots: 0..5 = p^T, 6..11 = o1^T, 12..17 = o2^T
            cT_sb = spool.tile([128, KC, BL], F32)

            # p^T = W_cls^T @ pooled^T + b_cls x 1 (runs during the h stream)
            for mt in range(HC):
                pw = ps2.tile([128, 16], F32, tag="ps2")
                for kc in range(HC):
                    nc.tensor.matmul(pw[:, 0:BL],
                                     wcls_sb[:, kc, mt * 128:(mt + 1) * 128],
                                     pooledT[:, kc, :],
                                     start=(kc == 0), stop=False)
                nc.tensor.matmul(pw[:, 0:BL],
                                 bcls_sb[0:1, mt * 128:(mt + 1) * 128],
                                 ones128[0:1, 0:BL], start=False, stop=True)
                nc.vector.tensor_copy(out=cT_sb[:, mt, :], in_=pw[:, 0:BL])

            # t12 = tanh(e12 / count)
            t12 = spool.tile([16, H], F32)
            nc.scalar.activation(out=t12[:, 0:512], in_=pe1, func=AF.Tanh,
                                 scale=recip)
            nc.scalar.activation(out=t12[:, 512:H], in_=pe2, func=AF.Tanh,
                                 scale=recip)

            t12T = spool.tile([128, HC, BL, 2], F32)
            for hc in range(HC):
                pt = ps2.tile([128, 16], F32, tag="ps2")
                nc.tensor.transpose(pt, t12[:, hc * 128:(hc + 1) * 128],
                                    ident[:16, :16])
                nc.vector.tensor_copy(out=t12T[:, hc], in_=pt)

            # o1^T / o2^T
            for j, (w_sb, b_sb) in enumerate(((we1_sb, be1_sb), (we2_sb, be2_sb))):
                for mt in range(HC):
                    pw = ps2.tile([128, 16], F32, tag="ps2")
                    for kc in range(HC):
                        nc.tensor.matmul(pw[:, 0:BL],
                                         w_sb[:, kc, mt * 128:(mt + 1) * 128],
                                         t12T[:, kc, :, j],
                                         start=(kc == 0), stop=False)
                    nc.tensor.matmul(pw[:, 0:BL],
                                     b_sb[0:1, mt * 128:(mt + 1) * 128],
                                     ones128[0:1, 0:BL], start=False, stop=True)
                    nc.vector.tensor_copy(out=cT_sb[:, HC * (1 + j) + mt, :],
                                          in_=pw[:, 0:BL])

            # logits = concat^T.T @ W_cat + b_cat
            plog = ps1.tile([BL, L], F32, tag="log")
            for s in range(KC):
                nc.tensor.matmul(plog, cT_sb[:, s, :], wcat_sb[:, s, :],
                                 start=(s == 0), stop=False)
            nc.tensor.matmul(plog, ones128[0:1, 0:BL], bcat_sb[0:1, :],
                             start=False, stop=True)

            # probs = softmax(logits)
            out_sb = spool.tile([BL, L + 1], F32)
            mx = spool.tile([BL, 1], F32)
            nc.vector.reduce_max(out=mx, in_=plog, axis=mybir.AxisListType.X)
            nmx = spool.tile([BL, 1], F32)
            nc.vector.tensor_scalar_mul(nmx, mx, -1.0)
            esb = spool.tile([BL, L], F32)
            nc.scalar.activation(out=esb, in_=plog, func=AF.Exp, bias=nmx)
            ssb = spool.tile([BL, 1], F32)
            nc.vector.reduce_sum(out=ssb, in_=esb, axis=mybir.AxisListType.X)
            rs = spool.tile([BL, 1], F32)
            nc.vector.reciprocal(rs, ssb)
            nc.vector.tensor_scalar_mul(out_sb[:, 0:L], esb, rs)

            # per-example -loss contribution: log_softmax(probs)[label]
            mx2 = spool.tile([BL, 1], F32)
            nc.vector.reduce_max(out=mx2, in_=out_sb[:, 0:L], axis=mybir.AxisListType.X)
            nmx2 = spool.tile([BL, 1], F32)
            nc.vector.tensor_scalar_mul(nmx2, mx2, -1.0)
            e2sb = spool.tile([BL, L], F32)
            nc.scalar.activation(out=e2sb, in_=out_sb[:, 0:L], func=AF.Exp, bias=nmx2)
            s2sb = spool.tile([BL, 1], F32)
            nc.vector.reduce_sum(out=s2sb, in_=e2sb, axis=mybir.AxisListType.X)
            ln2 = spool.tile([BL, 1], F32)
            nc.scalar.activation(out=ln2, in_=s2sb, func=AF.Ln)

            labf = spool.tile([BL, 1], F32)
            nc.vector.tensor_copy(out=labf, in_=lab_sb)
            onehot = spool.tile([BL, L], F32)
            nc.vector.tensor_scalar(out=onehot, in0=iota30, scalar1=labf,
                                    scalar2=None, op0=OP.is_equal)
            pk30 = spool.tile([BL, L], F32)
            nc.vector.tensor_mul(pk30, out_sb[:, 0:L], onehot)
            pk = spool.tile([BL, 1], F32)
            nc.vector.reduce_sum(out=pk, in_=pk30, axis=mybir.AxisListType.X)
            tsum = spool.tile([BL, 1], F32)
            nc.vector.tensor_add(tsum, pk, nmx2)
            nc.vector.tensor_sub(out_sb[:, L:L + 1], tsum, ln2)

            nc.sync.dma_start(out=out_d[:], in_=out_sb)

    nc.compile()
    return nc


def get_nc():
    if "nc" not in _CACHE:
        _CACHE["nc"] = build_nc()
    return _CACHE["nc"]


def make_in_maps(inputs):
    f32 = lambda x: np.ascontiguousarray(np.asarray(x), dtype=np.float32)
    i32 = lambda x: np.ascontiguousarray(np.asarray(x), dtype=np.int32)
    h = f32(inputs["last_hidden_states"])
    pooled = f32(inputs["pooled_output"])
    ent = i32(inputs["ent_ids"])
    labels = i32(inputs["labels"]).reshape(B, 1)
    w_cls = f32(inputs["W_cls"])
    w_e1 = f32(inputs["W_e1"])
    w_e2 = f32(inputs["W_e2"])
    w_cat = f32(inputs["W_cat"])
    b_cls = f32(inputs["b_cls"]).reshape(1, H)
    b_e1 = f32(inputs["b_e1"]).reshape(1, H)
    b_e2 = f32(inputs["b_e2"]).reshape(1, H)
    b_cat = f32(inputs["b_cat"]).reshape(1, L)

    in_maps = []
    for c in range(NCORES):
        sl = slice(c * BL, (c + 1) * BL)
        in_maps.append({
            "hid": h[sl], "pooled": pooled[sl], "ent": ent[sl],
            "labels": labels[sl],
            "w_cls": w_cls, "w_e1": w_e1, "w_e2": w_e2,
            "b_cls": b_cls, "b_e1": b_e1, "b_e2": b_e2,
            "w_cat": w_cat, "b_cat": b_cat,
            "zeros": np.zeros((128, TC * BL * 2 * BL), dtype=np.float32),
        })
    return in_maps


def postprocess(outs):
    probs = np.concatenate([o[:, 0:L] for o in outs], axis=0).astype(np.float32)
    lossp = np.concatenate([o[:, L] for o in outs], axis=0)
    loss = np.asarray(-lossp.astype(np.float64).mean(), dtype=np.float32)
    return probs, loss


def kernel(**inputs):
    from concourse import bass_utils
    nc = get_nc()
    res = bass_utils.run_bass_kernel_spmd(nc, make_in_maps(inputs),
                                          core_ids=list(range(NCORES)))
    return postprocess([r["out"] for r in res.results])


# revision 28
# speedup vs baseline: 1.1243x; 1.1243x over previous
"""Trainium2 Bass kernel for the roberta entity-span classification head.

Problem: nn_R_roberta_70360154243670 (segment_reduce, memory-bound).

  e1, e2 = per-example mean of last_hidden_states over the 1st / 2nd
           contiguous run of 1s in ent_ids
  p  = pooled @ W_cls + b_cls
  o1 = tanh(e1) @ W_e1 + b_e1 ; o2 = tanh(e2) @ W_e2 + b_e2
  logits = [p|o1|o2] @ W_cat + b_cat
  probs  = softmax(logits); loss = -mean(log_softmax(probs)[label])

Sharding: pure data parallel, batch 64 -> 8 cores x 8 examples,
small weights replicated to every core (host passes them transposed --
layout prep only, all FLOPs stay on device).

Device-side algorithm (per core, BL=8 examples):
  * span ids from ent_ids: starts = (ent[t] > ent[t-1]); inclusive cumsum
    over T via a triangular-ones matmul in [T-partition, batch-free]
    layout; span = cumsum * ent; m1 = (span==1), m2 = (span==2).
  * masked segment sums: one [16,768] PSUM accumulation over 32 matmuls
    (8 examples x 4 T-chunks) with zero-padded block-column masks as
    lhsT (fp32r: 1 cycle/row) streaming each h tile straight from DMA.
  * counts via ones-matmul -> 1/max(c,1) -> tanh(e * recip) on ACT.
  * while the h stream saturates DMA, the idle TensorE folds the weight
    chain:  Wc_j = W_j @ W_cat_j  [768,30] and
    bias_eff = b_cls@Wcat_0 + b_e1@Wcat_1 + b_e2@Wcat_2 + b_cat, so the
    post-stream tail is only tanh -> 6 transposes -> 19 small matmuls ->
    softmax/loss.
  * softmax + log-softmax + label pick on device; host only concatenates
    per-core outputs and averages 64 scalars.
"""

import numpy as np

import concourse.bass as bass
import concourse.mybir as mybir
import concourse.tile as tile
from concourse import bacc
from concourse.masks import make_identity, make_upper_triangular

B, T, H, L = 64, 512, 768, 30
NCORES = 8
BL = B // NCORES          # 8 examples per core
TC = T // 128             # 4 T-chunks
HC = H // 128             # 6 H-chunks

F32 = mybir.dt.float32
F32R = mybir.dt.float32r
I32 = mybir.dt.int32
AF = mybir.ActivationFunctionType
OP = mybir.AluOpType

_CACHE = {}


def build_nc():
    nc = bacc.Bacc("TRN2", target_bir_lowering=False)

    hid_d = nc.dram_tensor("hid", [BL, T, H], F32, kind="ExternalInput")
    pooled_d = nc.dram_tensor("pooled", [BL, H], F32, kind="ExternalInput")
    ent_d = nc.dram_tensor("ent", [BL, T], I32, kind="ExternalInput")
    lab_d = nc.dram_tensor("labels", [BL, 1], I32, kind="ExternalInput")
    # weight matrices arrive host-transposed and chunk-tiled:
    # wT_x[p, c, k] = W_x[k, c*128+p]  (partition-contiguous DMA layout)
    wTcls_d = nc.dram_tensor("wT_cls", [128, HC, H], F32, kind="ExternalInput")
    wTe1_d = nc.dram_tensor("wT_e1", [128, HC, H], F32, kind="ExternalInput")
    wTe2_d = nc.dram_tensor("wT_e2", [128, HC, H], F32, kind="ExternalInput")
    # bias vectors as one [H, 3] block of columns (b_cls | b_e1 | b_e2)
    b3_d = nc.dram_tensor("b3", [H, 3], F32, kind="ExternalInput")
    # w_cat chunk-tiled likewise: w_cat[p, c, l] = W_cat[c*128+p, l]
    wcat_d = nc.dram_tensor("w_cat", [128, 3 * HC, L], F32, kind="ExternalInput")
    bcat_d = nc.dram_tensor("b_cat", [1, L], F32, kind="ExternalInput")
    out_d = nc.dram_tensor("out", [BL, L + 1], F32, kind="ExternalOutput")

    with tile.TileContext(nc) as tc:
        with (
            tc.tile_pool(name="const", bufs=1) as cpool,
            tc.tile_pool(name="wpool", bufs=1) as wpool,
            tc.tile_pool(name="hpool", bufs=8) as hpool,
            tc.tile_pool(name="spool", bufs=1) as spool,
            tc.tile_pool(name="ps2", bufs=3, space="PSUM") as ps2,
            tc.tile_pool(name="ps1", bufs=1, space="PSUM") as ps1,
            tc.tile_pool(name="pse", bufs=1, space="PSUM") as pse,
        ):
            # ---- first h tile DMAs lead the sync-ring queue ----
            # T-chunk granularity so segment matmuls start per 393KB tile
            hv = hid_d[:].rearrange("b (c p) d -> p b c d", p=128)
            ht0 = [hpool.tile([128, H], F32R, tag="h", name=f"ht0_{c}")
                   for c in range(TC)]
            for c in range(TC):
                nc.sync.dma_start(out=ht0[c], in_=hv[:, 0, c].bitcast(F32R))

            # ---- constants built on device ----
            ones128 = cpool.tile([128, 128], F32)
            nc.vector.memset(ones128, 1.0)
            # dummy activations pull the ACT LUT table loads to t=0; order
            # matters: last-loaded set must cover Tanh+Exp (exp_and_others)
            # so the real tanh/exp need no further table swap.
            warm = cpool.tile([1, 4], F32)
            nc.vector.memset(warm, 1.0)
            nc.scalar.activation(out=warm[:, 1:2], in_=warm[:, 0:1], func=AF.Ln)
            nc.scalar.activation(out=warm[:, 2:3], in_=warm[:, 0:1], func=AF.Exp)
            nc.scalar.activation(out=warm[:, 3:4], in_=warm[:, 0:1], func=AF.Tanh)
            triu = cpool.tile([128, 128], F32)
            make_upper_triangular(nc, triu, val=1.0, diag=True)
            ident = cpool.tile([128, 128], F32)
            make_identity(nc, ident)
            iota30 = cpool.tile([BL, L], F32)
            nc.gpsimd.iota(iota30, pattern=[[1, L]], base=0,
                           channel_multiplier=0,
                           allow_small_or_imprecise_dtypes=True)

            # ---- small input DMAs on the ACT HWDGE ring (parallel with h) ----
            ent_sb = spool.tile([BL, T], I32)
            nc.scalar.dma_start(out=ent_sb, in_=ent_d[:])
            pooled_sb = spool.tile([BL, H], F32)
            nc.scalar.dma_start(out=pooled_sb, in_=pooled_d[:])
            lab_sb = spool.tile([BL, 1], I32)
            nc.scalar.dma_start(out=lab_sb, in_=lab_d[:])
            b3_sb = spool.tile([128, HC, 3], F32)
            nc.scalar.dma_start(out=b3_sb,
                                in_=b3_d[:].rearrange("(c p) o -> p c o", p=128))
            bcat_sb = spool.tile([1, L], F32)
            nc.scalar.dma_start(out=bcat_sb, in_=bcat_d[:])

            # ---- span-id mask pipeline (cheap, runs during first h DMA) ----
            entf = spool.tile([BL, T + 1], F32)
            nc.vector.memset(entf[:, 0:1], 0.0)
            nc.vector.tensor_copy(out=entf[:, 1:T + 1], in_=ent_sb)
            starts = spool.tile([BL, T], F32)
            nc.vector.tensor_tensor(out=starts, in0=entf[:, 1:T + 1],
                                    in1=entf[:, 0:T], op=OP.is_gt)

            startsT = spool.tile([128, TC, BL], F32)
            entT = spool.tile([128, TC, BL], F32)
            for c in range(TC):
                pt = ps2.tile([128, 16], F32, tag="ps2")
                nc.tensor.transpose(pt[:, 0:BL], starts[:, c * 128:(c + 1) * 128],
                                    ident[:BL, :BL])
                nc.vector.tensor_copy(out=startsT[:, c, :], in_=pt[:, 0:BL])
                pt2 = ps2.tile([128, 16], F32, tag="ps2")
                nc.tensor.transpose(pt2[:, 0:BL],
                                    entf[:, 1 + c * 128:1 + (c + 1) * 128],
                                    ident[:BL, :BL])
                nc.vector.tensor_copy(out=entT[:, c, :], in_=pt2[:, 0:BL])

            spanT = spool.tile([128, TC, BL], F32)
            for mc in range(TC):
                pc = ps2.tile([128, 16], F32, tag="ps2")
                for kc in range(mc + 1):
                    nc.tensor.matmul(pc[:, 0:BL],
                                     triu if kc == mc else ones128,
                                     startsT[:, kc, :],
                                     start=(kc == 0), stop=(kc == mc))
                nc.vector.tensor_mul(spanT[:, mc, :], pc[:, 0:BL], entT[:, mc, :])

            masks = spool.tile([128, TC, BL, 2], F32)
            for c in range(TC):
                nc.vector.tensor_scalar(out=masks[:, c, :, 0], in0=spanT[:, c, :],
                                        scalar1=1.0, scalar2=None, op0=OP.is_equal)
                nc.vector.tensor_scalar(out=masks[:, c, :, 1], in0=spanT[:, c, :],
                                        scalar1=2.0, scalar2=None, op0=OP.is_equal)

            # zero-padded block-column masks: col 2b+j of masksZ[:,c,b,:] holds
            # mask j of example b, other columns zero, so each (b,c) matmul
            # accumulates only into rows 2b:2b+2 of the shared [16,768] psum.
            # zero background produced on device: memset cannot encode f32r,
            # but a DVE TensorCopy with f32r output can (f32 -> f32r rounding)
            zf32 = spool.tile([128, TC * BL * 2 * BL], F32)
            nc.vector.memset(zf32, 0.0)
            masksZ = spool.tile([128, TC, BL, 2 * BL], F32R)
            nc.vector.tensor_copy(
                out=masksZ.rearrange("p c b k -> p (c b k)"), in_=zf32)
            for c in range(TC):
                for b in range(BL):
                    nc.vector.tensor_copy(out=masksZ[:, c, b, 2 * b:2 * b + 2],
                                          in_=masks[:, c, b, :])

            # counts -> 1/max(c,1), rows 2b+j
            pcnt = ps1.tile([16, 1], F32, tag="cnt")
            for c in range(TC):
                nc.tensor.matmul(pcnt, masks[:, c], ones128[:, 0:1],
                                 start=(c == 0), stop=(c == TC - 1))
            cnt_sb = spool.tile([16, 1], F32)
            nc.vector.tensor_scalar_max(cnt_sb, pcnt, 1.0)
            recip = spool.tile([16, 1], F32)
            nc.vector.reciprocal(recip, cnt_sb)

            # pooled transposed early (used by the logits matmuls at the tail)
            pooledT = spool.tile([128, HC, BL], F32)
            for hc in range(HC):
                pp = ps2.tile([128, 16], F32, tag="ps2")
                nc.tensor.transpose(pp[:, 0:BL],
                                    pooled_sb[:, hc * 128:(hc + 1) * 128],
                                    ident[:BL, :BL])
                nc.vector.tensor_copy(out=pooledT[:, hc, :], in_=pp[:, 0:BL])

            # ---- weight DMAs (interleaved with h stream below) ----
            wcat_sb = wpool.tile([128, 3 * HC, L], F32)
            wT_sb = [wpool.tile([128, HC, H], F32, tag=f"wT{j}", name=f"wT{j}")
                     for j in range(3)]
            wT_d = [wTcls_d, wTe1_d, wTe2_d]

            # ---- h stream + masked segment-sum matmuls (fp32r) ----
            pe1 = pse.tile([16, 512], F32, tag="pe1")
            pe2 = pse.tile([16, 256], F32, tag="pe2")
            for b in range(BL):
                for c in range(TC):
                    if b == 0:
                        ht = ht0[c]
                    else:
                        ht = hpool.tile([128, H], F32R, tag="h")
                        nc.sync.dma_start(out=ht, in_=hv[:, b, c].bitcast(F32R))
                    lhsT = masksZ[:, c, b, :]
                    nc.tensor.matmul(pe1, lhsT, ht[:, 0:512],
                                     start=(b == 0 and c == 0),
                                     stop=(b == BL - 1 and c == TC - 1))
                    nc.tensor.matmul(pe2, lhsT, ht[:, 512:H],
                                     start=(b == 0 and c == 0),
                                     stop=(b == BL - 1 and c == TC - 1))
                # weight loads threaded between example streams: on-chip well
                # before the fold matmuls need them, without delaying h tiles
                if b == 1:
                    nc.sync.dma_start(out=wcat_sb, in_=wcat_d[:])
                if b in (2, 3, 4):
                    j = b - 2
                    nc.sync.dma_start(out=wT_sb[j], in_=wT_d[j][:])

            # ---- weight-chain folding on the otherwise idle TensorE ----
            # Wc_j[k, l] = sum_m W_j[k, m] Wcat_j[m, l]  (lhsT = W_j^T natural)
            wc_sb = [wpool.tile([128, HC, L], F32, tag=f"wc{j}", name=f"wc{j}")
                     for j in range(3)]
            for j in range(3):
                for kc in range(HC):
                    pf = ps2.tile([128, 32], F32, tag="ps2")
                    for mc in range(HC):
                        nc.tensor.matmul(pf[:, 0:L], wT_sb[j][:, mc, kc * 128:(kc + 1) * 128],
                                         wcat_sb[:, j * HC + mc, :],
                                         start=(mc == 0), stop=(mc == HC - 1))
                    nc.vector.tensor_copy(out=wc_sb[j][:, kc, :], in_=pf[:, 0:L])

            # bias_eff = b_cls@Wcat_0 + b_e1@Wcat_1 + b_e2@Wcat_2 + b_cat
            pb = ps1.tile([16, 32], F32, tag="cnt")
            for j in range(3):
                for mc in range(HC):
                    nc.tensor.matmul(pb[0:1, 0:L], b3_sb[:, mc, j:j + 1],
                                     wcat_sb[:, j * HC + mc, :],
                                     start=(j == 0 and mc == 0), stop=False)
            nc.tensor.matmul(pb[0:1, 0:L], ones128[0:1, 0:1], bcat_sb[0:1, :],
                             start=False, stop=True)
            beff_sb = spool.tile([1, L], F32)
            nc.vector.tensor_copy(out=beff_sb, in_=pb[0:1, 0:L])

            # ---- tail: tanh, transpose, 19 small matmuls, softmax, loss ----
            t12 = spool.tile([16, H], F32)
            nc.scalar.activation(out=t12[:, 0:512], in_=pe1, func=AF.Tanh,
                                 scale=recip)
            nc.scalar.activation(out=t12[:, 512:H], in_=pe2, func=AF.Tanh,
                                 scale=recip)

            t12T = spool.tile([128, HC, BL, 2], F32)
            for hc in range(HC):
                pt = ps2.tile([128, 16], F32, tag="ps2")
                nc.tensor.transpose(pt, t12[:, hc * 128:(hc + 1) * 128],
                                    ident[:16, :16])
                nc.vector.tensor_copy(out=t12T[:, hc], in_=pt)

            # logits = pooled@Wc0 + t1@Wc1 + t2@Wc2 + bias_eff
            plog = ps1.tile([BL, L], F32, tag="log")
            n_mm = 3 * HC
            i = 0
            for kc in range(HC):
                nc.tensor.matmul(plog, pooledT[:, kc, :], wc_sb[0][:, kc, :],
                                 start=(i == 0), stop=False)
                i += 1
            for j in (1, 2):
                for kc in range(HC):
                    nc.tensor.matmul(plog, t12T[:, kc, :, j - 1],
                                     wc_sb[j][:, kc, :],
                                     start=False, stop=False)
                    i += 1
            nc.tensor.matmul(plog, ones128[0:1, 0:BL], beff_sb[0:1, :],
                             start=False, stop=True)

            # probs = softmax(logits)
            out_sb = spool.tile([BL, L + 1], F32)
            mx = spool.tile([BL, 1], F32)
            nc.vector.reduce_max(out=mx, in_=plog, axis=mybir.AxisListType.X)
            nmx = spool.tile([BL, 1], F32)
            nc.vector.tensor_scalar_mul(nmx, mx, -1.0)
            esb = spool.tile([BL, L], F32)
            nc.scalar.activation(out=esb, in_=plog, func=AF.Exp, bias=nmx)
            ssb = spool.tile([BL, 1], F32)
            nc.vector.reduce_sum(out=ssb, in_=esb, axis=mybir.AxisListType.X)
            rs = spool.tile([BL, 1], F32)
            nc.vector.reciprocal(rs, ssb)
            nc.vector.tensor_scalar_mul(out_sb[:, 0:L], esb, rs)

            # per-example -loss contribution: log_softmax(probs)[label]
            mx2 = spool.tile([BL, 1], F32)
            nc.vector.reduce_max(out=mx2, in_=out_sb[:, 0:L], axis=mybir.AxisListType.X)
            nmx2 = spool.tile([BL, 1], F32)
            nc.vector.tensor_scalar_mul(nmx2, mx2, -1.0)
            e2sb = spool.tile([BL, L], F32)
            nc.scalar.activation(out=e2sb, in_=out_sb[:, 0:L], func=AF.Exp, bias=nmx2)
            s2sb = spool.tile([BL, 1], F32)
            nc.vector.reduce_sum(out=s2sb, in_=e2sb, axis=mybir.AxisListType.X)
            ln2 = spool.tile([BL, 1], F32)
            nc.scalar.activation(out=ln2, in_=s2sb, func=AF.Ln)

            labf = spool.tile([BL, 1], F32)
            nc.vector.tensor_copy(out=labf, in_=lab_sb)
            onehot = spool.tile([BL, L], F32)
            nc.vector.tensor_scalar(out=onehot, in0=iota30, scalar1=labf,
                                    scalar2=None, op0=OP.is_equal)
            pk30 = spool.tile([BL, L], F32)
            nc.vector.tensor_mul(pk30, out_sb[:, 0:L], onehot)
            pk = spool.tile([BL, 1], F32)
            nc.vector.reduce_sum(out=pk, in_=pk30, axis=mybir.AxisListType.X)
            tsum = spool.tile([BL, 1], F32)
            nc.vector.tensor_add(tsum, pk, nmx2)
            nc.vector.tensor_sub(out_sb[:, L:L + 1], tsum, ln2)

            nc.sync.dma_start(out=out_d[:], in_=out_sb)

    nc.compile()
    return nc


def get_nc():
    if "nc" not in _CACHE:
        _CACHE["nc"] = build_nc()
    return _CACHE["nc"]


def make_in_maps(inputs):
    f32 = lambda x: np.ascontiguousarray(np.asarray(x), dtype=np.float32)
    i32 = lambda x: np.ascontiguousarray(np.asarray(x), dtype=np.int32)
    h = f32(inputs["last_hidden_states"])
    pooled = f32(inputs["pooled_output"])
    ent = i32(inputs["ent_ids"])
    labels = i32(inputs["labels"]).reshape(B, 1)
    def chunk_tile(mat):  # [R, C] -> [128, R//128, C] partition-contiguous
        r, c = mat.shape
        return np.ascontiguousarray(
            mat.reshape(r // 128, 128, c).transpose(1, 0, 2))

    wT_cls = chunk_tile(f32(np.asarray(inputs["W_cls"]).T))
    wT_e1 = chunk_tile(f32(np.asarray(inputs["W_e1"]).T))
    wT_e2 = chunk_tile(f32(np.asarray(inputs["W_e2"]).T))
    w_cat = chunk_tile(f32(inputs["W_cat"]))
    b3 = np.stack([f32(inputs["b_cls"]), f32(inputs["b_e1"]),
                   f32(inputs["b_e2"])], axis=1)
    b3 = np.ascontiguousarray(b3, dtype=np.float32)
    b_cat = f32(inputs["b_cat"]).reshape(1, L)

    in_maps = []
    for c in range(NCORES):
        sl = slice(c * BL, (c + 1) * BL)
        in_maps.append({
            "hid": h[sl], "pooled": pooled[sl], "ent": ent[sl],
            "labels": labels[sl],
            "wT_cls": wT_cls, "wT_e1": wT_e1, "wT_e2": wT_e2,
            "b3": b3, "w_cat": w_cat, "b_cat": b_cat,
        })
    return in_maps


def postprocess(outs):
    probs = np.concatenate([o[:, 0:L] for o in outs], axis=0).astype(np.float32)
    lossp = np.concatenate([o[:, L] for o in outs], axis=0)
    loss = np.asarray(-lossp.astype(np.float64).mean(), dtype=np.float32)
    return probs, loss


def kernel(**inputs):
    from concourse import bass_utils
    nc = get_nc()
    res = bass_utils.run_bass_kernel_spmd(nc, make_in_maps(inputs),
                                          core_ids=list(range(NCORES)))
    return postprocess([r["out"] for r in res.results])


# revision 38
# speedup vs baseline: 1.1402x; 1.0142x over previous
"""Trainium2 Bass kernel for the roberta entity-span classification head.

Problem: nn_R_roberta_70360154243670 (segment_reduce, memory-bound).

  e1, e2 = per-example mean of last_hidden_states over the 1st / 2nd
           contiguous run of 1s in ent_ids
  p  = pooled @ W_cls + b_cls
  o1 = tanh(e1) @ W_e1 + b_e1 ; o2 = tanh(e2) @ W_e2 + b_e2
  logits = [p|o1|o2] @ W_cat + b_cat
  probs  = softmax(logits); loss = -mean(log_softmax(probs)[label])

Sharding: pure data parallel, batch 64 -> 8 cores x 8 examples,
small weights replicated to every core (host passes them transposed --
layout prep only, all FLOPs stay on device).

Device-side algorithm (per core, BL=8 examples):
  * span ids from ent_ids: starts = (ent[t] > ent[t-1]); inclusive cumsum
    over T via a triangular-ones matmul in [T-partition, batch-free]
    layout; span = cumsum * ent; m1 = (span==1), m2 = (span==2).
  * masked segment sums: one [16,768] PSUM accumulation over 32 matmuls
    (8 examples x 4 T-chunks) with zero-padded block-column masks as
    lhsT (fp32r: 1 cycle/row) streaming each h tile straight from DMA.
  * counts via ones-matmul -> 1/max(c,1) -> tanh(e * recip) on ACT.
  * while the h stream saturates DMA, the idle TensorE folds the weight
    chain:  Wc_j = W_j @ W_cat_j  [768,30] and
    bias_eff = b_cls@Wcat_0 + b_e1@Wcat_1 + b_e2@Wcat_2 + b_cat, so the
    post-stream tail is only tanh -> 6 transposes -> 19 small matmuls ->
    softmax/loss.
  * softmax + log-softmax + label pick on device; host only concatenates
    per-core outputs and averages 64 scalars.
"""

import numpy as np

import concourse.bass as bass
import concourse.mybir as mybir
import concourse.tile as tile
from concourse import bacc
from concourse.masks import make_identity, make_upper_triangular

B, T, H, L = 64, 512, 768, 30
NCORES = 8
BL = B // NCORES          # 8 examples per core
TC = T // 128             # 4 T-chunks
HC = H // 128             # 6 H-chunks

F32 = mybir.dt.float32
F32R = mybir.dt.float32r
I32 = mybir.dt.int32
AF = mybir.ActivationFunctionType
OP = mybir.AluOpType

_CACHE = {}


def build_nc():
    nc = bacc.Bacc("TRN2", target_bir_lowering=False)

    hid_d = nc.dram_tensor("hid", [BL, T, H], F32, kind="ExternalInput")
    pooled_d = nc.dram_tensor("pooled", [BL, H], F32, kind="ExternalInput")
    ent_d = nc.dram_tensor("ent", [BL, T], I32, kind="ExternalInput")
    lab_d = nc.dram_tensor("labels", [BL, 1], I32, kind="ExternalInput")
    # weight matrices arrive host-transposed and chunk-tiled:
    # wT_x[p, c, k] = W_x[k, c*128+p]  (partition-contiguous DMA layout)
    wTcls_d = nc.dram_tensor("wT_cls", [128, HC, H], F32, kind="ExternalInput")
    wTe1_d = nc.dram_tensor("wT_e1", [128, HC, H], F32, kind="ExternalInput")
    wTe2_d = nc.dram_tensor("wT_e2", [128, HC, H], F32, kind="ExternalInput")
    # bias vectors as one [H, 3] block of columns (b_cls | b_e1 | b_e2)
    b3_d = nc.dram_tensor("b3", [H, 3], F32, kind="ExternalInput")
    # w_cat chunk-tiled likewise: w_cat[p, c, l] = W_cat[c*128+p, l]
    wcat_d = nc.dram_tensor("w_cat", [128, 3 * HC, L], F32, kind="ExternalInput")
    bcat_d = nc.dram_tensor("b_cat", [1, L], F32, kind="ExternalInput")
    out_d = nc.dram_tensor("out", [BL, L + 1], F32, kind="ExternalOutput")

    with tile.TileContext(nc) as tc:
        with (
            tc.tile_pool(name="const", bufs=1) as cpool,
            tc.tile_pool(name="wpool", bufs=1) as wpool,
            tc.tile_pool(name="hpool", bufs=12) as hpool,
            tc.tile_pool(name="spool", bufs=1) as spool,
            tc.tile_pool(name="ps2", bufs=3, space="PSUM") as ps2,
            tc.tile_pool(name="ps1", bufs=1, space="PSUM") as ps1,
            tc.tile_pool(name="pse", bufs=1, space="PSUM") as pse,
        ):
            # ---- first h tile DMAs lead the sync-ring queue ----
            # T-chunk granularity so segment matmuls start per 393KB tile
            hv = hid_d[:].rearrange("b (c p) d -> p b c d", p=128)
            ht0 = [hpool.tile([128, H], F32R, tag="h", name=f"ht0_{c}")
                   for c in range(TC)]
            for c in range(TC):
                nc.sync.dma_start(out=ht0[c], in_=hv[:, 0, c].bitcast(F32R))

            # ---- constants built on device ----
            ones128 = cpool.tile([128, 128], F32)
            nc.vector.memset(ones128, 1.0)
            # dummy activations pull the ACT LUT table loads to t=0; order
            # matters: last-loaded set must cover Tanh+Exp (exp_and_others)
            # so the real tanh/exp need no further table swap.
            warm = cpool.tile([1, 4], F32)
            nc.vector.memset(warm, 1.0)
            nc.scalar.activation(out=warm[:, 2:3], in_=warm[:, 0:1], func=AF.Exp)
            nc.scalar.activation(out=warm[:, 3:4], in_=warm[:, 0:1], func=AF.Tanh)
            triu = cpool.tile([128, 128], F32)
            make_upper_triangular(nc, triu, val=1.0, diag=True)
            ident = cpool.tile([128, 128], F32)
            make_identity(nc, ident)
            iota30 = cpool.tile([BL, L], F32)
            nc.gpsimd.iota(iota30, pattern=[[1, L]], base=0,
                           channel_multiplier=0,
                           allow_small_or_imprecise_dtypes=True)

            # ---- small input DMAs on the ACT HWDGE ring (parallel with h) ----
            ent_sb = spool.tile([BL, T], I32)
            nc.scalar.dma_start(out=ent_sb, in_=ent_d[:])
            pooled_sb = spool.tile([BL, H], F32)
            nc.scalar.dma_start(out=pooled_sb, in_=pooled_d[:])
            lab_sb = spool.tile([BL, 1], I32)
            nc.scalar.dma_start(out=lab_sb, in_=lab_d[:])
            b3_sb = spool.tile([128, HC, 3], F32)
            nc.scalar.dma_start(out=b3_sb,
                                in_=b3_d[:].rearrange("(c p) o -> p c o", p=128))
            bcat_sb = spool.tile([1, L], F32)
            nc.scalar.dma_start(out=bcat_sb, in_=bcat_d[:])

            # ---- span-id mask pipeline (cheap, runs during first h DMA) ----
            entf = spool.tile([BL, T + 1], F32)
            nc.vector.memset(entf[:, 0:1], 0.0)
            nc.vector.tensor_copy(out=entf[:, 1:T + 1], in_=ent_sb)
            starts = spool.tile([BL, T], F32)
            nc.vector.tensor_tensor(out=starts, in0=entf[:, 1:T + 1],
                                    in1=entf[:, 0:T], op=OP.is_gt)

            startsT = spool.tile([128, TC, BL], F32)
            entT = spool.tile([128, TC, BL], F32)
            for c in range(TC):
                pt = ps2.tile([128, 16], F32, tag="ps2")
                nc.tensor.transpose(pt[:, 0:BL], starts[:, c * 128:(c + 1) * 128],
                                    ident[:BL, :BL])
                nc.vector.tensor_copy(out=startsT[:, c, :], in_=pt[:, 0:BL])
                pt2 = ps2.tile([128, 16], F32, tag="ps2")
                nc.tensor.transpose(pt2[:, 0:BL],
                                    entf[:, 1 + c * 128:1 + (c + 1) * 128],
                                    ident[:BL, :BL])
                nc.vector.tensor_copy(out=entT[:, c, :], in_=pt2[:, 0:BL])

            spanT = spool.tile([128, TC, BL], F32)
            for mc in range(TC):
                pc = ps2.tile([128, 16], F32, tag="ps2")
                for kc in range(mc + 1):
                    nc.tensor.matmul(pc[:, 0:BL],
                                     triu if kc == mc else ones128,
                                     startsT[:, kc, :],
                                     start=(kc == 0), stop=(kc == mc))
                nc.vector.tensor_mul(spanT[:, mc, :], pc[:, 0:BL], entT[:, mc, :])

            masks = spool.tile([128, TC, BL, 2], F32)
            for c in range(TC):
                nc.vector.tensor_scalar(out=masks[:, c, :, 0], in0=spanT[:, c, :],
                                        scalar1=1.0, scalar2=None, op0=OP.is_equal)
                nc.vector.tensor_scalar(out=masks[:, c, :, 1], in0=spanT[:, c, :],
                                        scalar1=2.0, scalar2=None, op0=OP.is_equal)

            # zero-padded block-column masks: col 2b+j of masksZ[:,c,b,:] holds
            # mask j of example b, other columns zero, so each (b,c) matmul
            # accumulates only into rows 2b:2b+2 of the shared [16,768] psum.
            # zero background produced on device: memset cannot encode f32r,
            # but a DVE TensorCopy with f32r output can (f32 -> f32r rounding)
            zf32 = spool.tile([128, TC * BL * 2 * BL], F32)
            nc.vector.memset(zf32, 0.0)
            masksZ = spool.tile([128, TC, BL, 2 * BL], F32R)
            nc.vector.tensor_copy(
                out=masksZ.rearrange("p c b k -> p (c b k)"), in_=zf32)
            for c in range(TC):
                for b in range(BL):
                    nc.vector.tensor_copy(out=masksZ[:, c, b, 2 * b:2 * b + 2],
                                          in_=masks[:, c, b, :])

            # counts -> 1/max(c,1), rows 2b+j
            pcnt = ps1.tile([16, 1], F32, tag="cnt")
            for c in range(TC):
                nc.tensor.matmul(pcnt, masks[:, c], ones128[:, 0:1],
                                 start=(c == 0), stop=(c == TC - 1))
            cnt_sb = spool.tile([16, 1], F32)
            nc.vector.tensor_scalar_max(cnt_sb, pcnt, 1.0)
            recip = spool.tile([16, 1], F32)
            nc.vector.reciprocal(recip, cnt_sb)

            # pooled transposed early (used by the logits matmuls at the tail)
            pooledT = spool.tile([128, HC, BL], F32)
            for hc in range(HC):
                pp = ps2.tile([128, 16], F32, tag="ps2")
                nc.tensor.transpose(pp[:, 0:BL],
                                    pooled_sb[:, hc * 128:(hc + 1) * 128],
                                    ident[:BL, :BL])
                nc.vector.tensor_copy(out=pooledT[:, hc, :], in_=pp[:, 0:BL])

            # ---- weight DMAs (interleaved with h stream below) ----
            wcat_sb = wpool.tile([128, 3 * HC, L], F32)
            wT_sb = [wpool.tile([128, HC, H], F32, tag=f"wT{j}", name=f"wT{j}")
                     for j in range(3)]
            wT_d = [wTcls_d, wTe1_d, wTe2_d]

            # ---- h stream + masked segment-sum matmuls (fp32r) ----
            pe1 = pse.tile([16, 512], F32, tag="pe1")
            pe2 = pse.tile([16, 256], F32, tag="pe2")
            for b in range(BL):
                for c in range(TC):
                    if b == 0:
                        ht = ht0[c]
                    else:
                        ht = hpool.tile([128, H], F32R, tag="h")
                        nc.sync.dma_start(out=ht, in_=hv[:, b, c].bitcast(F32R))
                    lhsT = masksZ[:, c, b, :]
                    nc.tensor.matmul(pe1, lhsT, ht[:, 0:512],
                                     start=(b == 0 and c == 0),
                                     stop=(b == BL - 1 and c == TC - 1))
                    nc.tensor.matmul(pe2, lhsT, ht[:, 512:H],
                                     start=(b == 0 and c == 0),
                                     stop=(b == BL - 1 and c == TC - 1))
                # weight loads threaded between example streams: on-chip well
                # before the fold matmuls need them, without delaying h tiles
                if b == 1:
                    nc.sync.dma_start(out=wcat_sb, in_=wcat_d[:])
                if b in (2, 3, 4):
                    j = b - 2
                    nc.sync.dma_start(out=wT_sb[j], in_=wT_d[j][:])

            # ---- weight-chain folding on the otherwise idle TensorE ----
            # Wc_j[k, l] = sum_m W_j[k, m] Wcat_j[m, l]  (lhsT = W_j^T natural)
            wc_sb = [wpool.tile([128, HC, L], F32, tag=f"wc{j}", name=f"wc{j}")
                     for j in range(3)]
            for j in range(3):
                for kc in range(HC):
                    pf = ps2.tile([128, 32], F32, tag="ps2")
                    for mc in range(HC):
                        nc.tensor.matmul(pf[:, 0:L], wT_sb[j][:, mc, kc * 128:(kc + 1) * 128],
                                         wcat_sb[:, j * HC + mc, :],
                                         start=(mc == 0), stop=(mc == HC - 1))
                    nc.vector.tensor_copy(out=wc_sb[j][:, kc, :], in_=pf[:, 0:L])

            # bias_eff = b_cls@Wcat_0 + b_e1@Wcat_1 + b_e2@Wcat_2 + b_cat
            pb = ps1.tile([16, 32], F32, tag="cnt")
            for j in range(3):
                for mc in range(HC):
                    nc.tensor.matmul(pb[0:1, 0:L], b3_sb[:, mc, j:j + 1],
                                     wcat_sb[:, j * HC + mc, :],
                                     start=(j == 0 and mc == 0), stop=False)
            nc.tensor.matmul(pb[0:1, 0:L], ones128[0:1, 0:1], bcat_sb[0:1, :],
                             start=False, stop=True)
            beff_sb = spool.tile([1, L], F32)
            nc.vector.tensor_copy(out=beff_sb, in_=pb[0:1, 0:L])

            # ---- tail: tanh, transpose, 19 small matmuls, softmax, loss ----
            t12 = spool.tile([16, H], F32)
            t12T = spool.tile([128, HC, BL, 2], F32)
            nc.scalar.activation(out=t12[:, 0:512], in_=pe1, func=AF.Tanh,
                                 scale=recip)
            nc.scalar.activation(out=t12[:, 512:H], in_=pe2, func=AF.Tanh,
                                 scale=recip)
            for hc in range(HC):
                pt = ps2.tile([128, 16], F32, tag="ps2")
                nc.tensor.transpose(pt, t12[:, hc * 128:(hc + 1) * 128],
                                    ident[:16, :16])
                nc.vector.tensor_copy(out=t12T[:, hc], in_=pt)

            # logits = pooled@Wc0 + t1@Wc1 + t2@Wc2 + bias_eff
            plog = ps1.tile([BL, L], F32, tag="log")
            n_mm = 3 * HC
            i = 0
            for kc in range(HC):
                nc.tensor.matmul(plog, pooledT[:, kc, :], wc_sb[0][:, kc, :],
                                 start=(i == 0), stop=False)
                i += 1
            for j in (1, 2):
                for kc in range(HC):
                    nc.tensor.matmul(plog, t12T[:, kc, :, j - 1],
                                     wc_sb[j][:, kc, :],
                                     start=False, stop=False)
                    i += 1
            nc.tensor.matmul(plog, ones128[0:1, 0:BL], beff_sb[0:1, :],
                             start=False, stop=True)

            # probs = softmax(logits)
            out_sb = spool.tile([BL, L + 1], F32)
            mx = spool.tile([BL, 1], F32)
            nc.vector.reduce_max(out=mx, in_=plog, axis=mybir.AxisListType.X)
            nmx = spool.tile([BL, 1], F32)
            nc.vector.tensor_scalar_mul(nmx, mx, -1.0)
            esb = spool.tile([BL, L], F32)
            ssb = spool.tile([BL, 1], F32)
            nc.scalar.activation(out=esb, in_=plog, func=AF.Exp, bias=nmx,
                                 accum_out=ssb)
            rs = spool.tile([BL, 1], F32)
            nc.vector.reciprocal(rs, ssb)
            nc.vector.tensor_scalar_mul(out_sb[:, 0:L], esb, rs)

            # per-example -loss contribution: log_softmax(probs)[label].
            # probs in [0,1] so exp(probs) needs no max-shift; exp(esb*rs)
            # fuses the normalize so it runs parallel with the probs mul.
            # ln computed on DVE via exponent extraction + atanh series
            # (avoids the 1.3us ACT natural_log table swap on the tail).
            e2sb = spool.tile([BL, L], F32)
            s2sb = spool.tile([BL, 1], F32)
            nc.scalar.activation(out=e2sb, in_=esb, func=AF.Exp, scale=rs,
                                 accum_out=s2sb)

            # ln(s2): s2 = m * 2^e, ln = e*ln2 + 2*atanh((m-1)/(m+1))
            bits = s2sb.bitcast(I32)
            e_i = spool.tile([BL, 1], I32)
            nc.vector.tensor_single_scalar(out=e_i, in_=bits, scalar=23,
                                           op=OP.arith_shift_right)
            e_f = spool.tile([BL, 1], F32)
            nc.vector.tensor_copy(out=e_f, in_=e_i)  # biased exponent e+127
            m_i = spool.tile([BL, 1], I32)
            nc.vector.tensor_scalar(out=m_i, in0=bits, scalar1=0x007FFFFF,
                                    scalar2=0x3F800000, op0=OP.bitwise_and,
                                    op1=OP.bitwise_or)
            m_f = m_i.bitcast(F32)               # mantissa in [1, 2)
            num = spool.tile([BL, 1], F32)
            nc.vector.tensor_scalar_sub(num, m_f, 1.0)
            den = spool.tile([BL, 1], F32)
            nc.vector.tensor_scalar_add(den, m_f, 1.0)
            rden = spool.tile([BL, 1], F32)
            nc.vector.reciprocal(rden, den)
            w = spool.tile([BL, 1], F32)
            nc.vector.tensor_mul(w, num, rden)   # w in [0, 1/3]
            w2 = spool.tile([BL, 1], F32)
            nc.vector.tensor_mul(w2, w, w)
            poly = spool.tile([BL, 1], F32)
            nc.vector.tensor_scalar(out=poly, in0=w2, scalar1=1.0 / 7.0,
                                    scalar2=1.0 / 5.0, op0=OP.mult, op1=OP.add)
            nc.vector.tensor_scalar(out=poly, in0=poly, scalar1=w2,
                                    scalar2=1.0 / 3.0, op0=OP.mult, op1=OP.add)
            nc.vector.tensor_scalar(out=poly, in0=poly, scalar1=w2,
                                    scalar2=1.0, op0=OP.mult, op1=OP.add)
            atanh = spool.tile([BL, 1], F32)
            nc.vector.tensor_mul(atanh, poly, w)  # atanh(w)
            eln2 = spool.tile([BL, 1], F32)
            nc.vector.tensor_scalar(out=eln2, in0=e_f,
                                    scalar1=float(np.log(2.0)),
                                    scalar2=float(127.0 * np.log(2.0)),
                                    op0=OP.mult, op1=OP.subtract)
            ln2t = spool.tile([BL, 1], F32)
            nc.vector.tensor_scalar(out=ln2t, in0=atanh, scalar1=2.0,
                                    scalar2=eln2, op0=OP.mult, op1=OP.add)

            labf = spool.tile([BL, 1], F32)
            nc.vector.tensor_copy(out=labf, in_=lab_sb)
            onehot = spool.tile([BL, L], F32)
            nc.vector.tensor_scalar(out=onehot, in0=iota30, scalar1=labf,
                                    scalar2=None, op0=OP.is_equal)
            pk30 = spool.tile([BL, L], F32)
            nc.vector.tensor_mul(pk30, out_sb[:, 0:L], onehot)
            pk = spool.tile([BL, 1], F32)
            nc.vector.reduce_sum(out=pk, in_=pk30, axis=mybir.AxisListType.X)
            nc.vector.tensor_sub(out_sb[:, L:L + 1], pk, ln2t)

            nc.sync.dma_start(out=out_d[:], in_=out_sb)

    nc.compile()
    return nc


def get_nc():
    if "nc" not in _CACHE:
        _CACHE["nc"] = build_nc()
    return _CACHE["nc"]


def make_in_maps(inputs):
    f32 = lambda x: np.ascontiguousarray(np.asarray(x), dtype=np.float32)
    i32 = lambda x: np.ascontiguousarray(np.asarray(x), dtype=np.int32)
    h = f32(inputs["last_hidden_states"])
    pooled = f32(inputs["pooled_output"])
    ent = i32(inputs["ent_ids"])
    labels = i32(inputs["labels"]).reshape(B, 1)
    def chunk_tile(mat):  # [R, C] -> [128, R//128, C] partition-contiguous
        r, c = mat.shape
        return np.ascontiguousarray(
            mat.reshape(r // 128, 128, c).transpose(1, 0, 2))

    wT_cls = chunk_tile(f32(np.asarray(inputs["W_cls"]).T))
    wT_e1 = chunk_tile(f32(np.asarray(inputs["W_e1"]).T))
    wT_e2 = chunk_tile(f32(np.asarray(inputs["W_e2"]).T))
    w_cat = chunk_tile(f32(inputs["W_cat"]))
    b3 = np.stack([f32(inputs["b_cls"]), f32(inputs["b_e1"]),
                   f32(inputs["b_e2"])], axis=1)
    b3 = np.ascontiguousarray(b3, dtype=np.float32)
    b_cat = f32(inputs["b_cat"]).reshape(1, L)

    in_maps = []
    for c in range(NCORES):
        sl = slice(c * BL, (c + 1) * BL)
        in_maps.append({
            "hid": h[sl], "pooled": pooled[sl], "ent": ent[sl],
            "labels": labels[sl],
            "wT_cls": wT_cls, "wT_e1": wT_e1, "wT_e2": wT_e2,
            "b3": b3, "w_cat": w_cat, "b_cat": b_cat,
        })
    return in_maps


def postprocess(outs):
    probs = np.concatenate([o[:, 0:L] for o in outs], axis=0).astype(np.float32)
    lossp = np.concatenate([o[:, L] for o in outs], axis=0)
    loss = np.asarray(-lossp.astype(np.float64).mean(), dtype=np.float32)
    return probs, loss


def kernel(**inputs):
    from concourse import bass_utils
    nc = get_nc()
    res = bass_utils.run_bass_kernel_spmd(nc, make_in_maps(inputs),
                                          core_ids=list(range(NCORES)))
    return postprocess([r["out"] for r in res.results])


# revision 42
# speedup vs baseline: 1.1426x; 1.0020x over previous
"""Trainium2 Bass kernel for the roberta entity-span classification head.

Problem: nn_R_roberta_70360154243670 (segment_reduce, memory-bound).

  e1, e2 = per-example mean of last_hidden_states over the 1st / 2nd
           contiguous run of 1s in ent_ids
  p  = pooled @ W_cls + b_cls
  o1 = tanh(e1) @ W_e1 + b_e1 ; o2 = tanh(e2) @ W_e2 + b_e2
  logits = [p|o1|o2] @ W_cat + b_cat
  probs  = softmax(logits); loss = -mean(log_softmax(probs)[label])

Sharding: pure data parallel, batch 64 -> 8 cores x 8 examples,
small weights replicated to every core (host passes them transposed --
layout prep only, all FLOPs stay on device).

Device-side algorithm (per core, BL=8 examples):
  * span ids from ent_ids: starts = (ent[t] > ent[t-1]); inclusive cumsum
    over T via a triangular-ones matmul in [T-partition, batch-free]
    layout; span = cumsum * ent; m1 = (span==1), m2 = (span==2).
  * masked segment sums: one [16,768] PSUM accumulation over 32 matmuls
    (8 examples x 4 T-chunks) with zero-padded block-column masks as
    lhsT (fp32r: 1 cycle/row) streaming each h tile straight from DMA.
  * counts via ones-matmul -> 1/max(c,1) -> tanh(e * recip) on ACT.
  * while the h stream saturates DMA, the idle TensorE folds the weight
    chain:  Wc_j = W_j @ W_cat_j  [768,30] and
    bias_eff = b_cls@Wcat_0 + b_e1@Wcat_1 + b_e2@Wcat_2 + b_cat, so the
    post-stream tail is only tanh -> 6 transposes -> 19 small matmuls ->
    softmax/loss.
  * softmax + log-softmax + label pick on device; host only concatenates
    per-core outputs and averages 64 scalars.
"""

import numpy as np

import concourse.bass as bass
import concourse.mybir as mybir
import concourse.tile as tile
from concourse import bacc
from concourse.masks import make_identity, make_upper_triangular

B, T, H, L = 64, 512, 768, 30
NCORES = 8
BL = B // NCORES          # 8 examples per core
TC = T // 128             # 4 T-chunks
HC = H // 128             # 6 H-chunks

F32 = mybir.dt.float32
F32R = mybir.dt.float32r
I32 = mybir.dt.int32
AF = mybir.ActivationFunctionType
OP = mybir.AluOpType

_CACHE = {}


def build_nc():
    nc = bacc.Bacc("TRN2", target_bir_lowering=False)

    hid_d = nc.dram_tensor("hid", [BL, T, H], F32, kind="ExternalInput")
    pooled_d = nc.dram_tensor("pooled", [BL, H], F32, kind="ExternalInput")
    ent_d = nc.dram_tensor("ent", [BL, T], I32, kind="ExternalInput")
    lab_d = nc.dram_tensor("labels", [BL, 1], I32, kind="ExternalInput")
    # weight matrices arrive host-transposed and chunk-tiled:
    # wT_x[p, c, k] = W_x[k, c*128+p]  (partition-contiguous DMA layout)
    wTcls_d = nc.dram_tensor("wT_cls", [128, HC, H], F32, kind="ExternalInput")
    wTe1_d = nc.dram_tensor("wT_e1", [128, HC, H], F32, kind="ExternalInput")
    wTe2_d = nc.dram_tensor("wT_e2", [128, HC, H], F32, kind="ExternalInput")
    # bias vectors as one [H, 3] block of columns (b_cls | b_e1 | b_e2)
    b3_d = nc.dram_tensor("b3", [H, 3], F32, kind="ExternalInput")
    # w_cat chunk-tiled likewise: w_cat[p, c, l] = W_cat[c*128+p, l]
    wcat_d = nc.dram_tensor("w_cat", [128, 3 * HC, L], F32, kind="ExternalInput")
    bcat_d = nc.dram_tensor("b_cat", [1, L], F32, kind="ExternalInput")
    out_d = nc.dram_tensor("out", [BL, L + 1], F32, kind="ExternalOutput")

    with tile.TileContext(nc) as tc:
        with (
            tc.tile_pool(name="const", bufs=1) as cpool,
            tc.tile_pool(name="wpool", bufs=1) as wpool,
            tc.tile_pool(name="hpool", bufs=12) as hpool,
            tc.tile_pool(name="spool", bufs=1) as spool,
            tc.tile_pool(name="ps2", bufs=4, space="PSUM") as ps2,
            tc.tile_pool(name="ps1", bufs=1, space="PSUM") as ps1,
            tc.tile_pool(name="pse", bufs=1, space="PSUM") as pse,
        ):
            # ---- first h tile DMAs lead the sync-ring queue ----
            # T-chunk granularity so segment matmuls start per 393KB tile
            hv = hid_d[:].rearrange("b (c p) d -> p b c d", p=128)
            ht0 = [hpool.tile([128, H], F32R, tag="h", name=f"ht0_{c}")
                   for c in range(TC)]
            for c in range(TC):
                nc.sync.dma_start(out=ht0[c], in_=hv[:, 0, c].bitcast(F32R))

            # ---- constants built on device ----
            ones128 = cpool.tile([128, 128], F32)
            nc.vector.memset(ones128, 1.0)
            # dummy activations pull the ACT LUT table loads to t=0; order
            # matters: last-loaded set must cover Tanh+Exp (exp_and_others)
            # so the real tanh/exp need no further table swap.
            warm = cpool.tile([1, 4], F32)
            nc.vector.memset(warm, 1.0)
            nc.scalar.activation(out=warm[:, 2:3], in_=warm[:, 0:1], func=AF.Exp)
            nc.scalar.activation(out=warm[:, 3:4], in_=warm[:, 0:1], func=AF.Tanh)
            triu = cpool.tile([128, 128], F32)
            make_upper_triangular(nc, triu, val=1.0, diag=True)
            ident = cpool.tile([128, 128], F32)
            make_identity(nc, ident)
            iota30 = cpool.tile([BL, L], F32)
            nc.gpsimd.iota(iota30, pattern=[[1, L]], base=0,
                           channel_multiplier=0,
                           allow_small_or_imprecise_dtypes=True)

            # ---- small input DMAs on the ACT HWDGE ring (parallel with h) ----
            ent_sb = spool.tile([BL, T], I32)
            nc.scalar.dma_start(out=ent_sb, in_=ent_d[:])
            pooled_sb = spool.tile([BL, H], F32)
            nc.scalar.dma_start(out=pooled_sb, in_=pooled_d[:])
            lab_sb = spool.tile([BL, 1], I32)
            nc.scalar.dma_start(out=lab_sb, in_=lab_d[:])
            b3_sb = spool.tile([128, HC, 3], F32)
            nc.scalar.dma_start(out=b3_sb,
                                in_=b3_d[:].rearrange("(c p) o -> p c o", p=128))
            bcat_sb = spool.tile([1, L], F32)
            nc.scalar.dma_start(out=bcat_sb, in_=bcat_d[:])

            # ---- span-id mask pipeline (cheap, runs during first h DMA) ----
            entf = spool.tile([BL, T + 1], F32)
            nc.vector.memset(entf[:, 0:1], 0.0)
            nc.vector.tensor_copy(out=entf[:, 1:T + 1], in_=ent_sb)
            starts = spool.tile([BL, T], F32)
            nc.vector.tensor_tensor(out=starts, in0=entf[:, 1:T + 1],
                                    in1=entf[:, 0:T], op=OP.is_gt)

            startsT = spool.tile([128, TC, BL], F32)
            entT = spool.tile([128, TC, BL], F32)
            for c in range(TC):
                pt = ps2.tile([128, 16], F32, tag="ps2")
                nc.tensor.transpose(pt[:, 0:BL], starts[:, c * 128:(c + 1) * 128],
                                    ident[:BL, :BL])
                nc.vector.tensor_copy(out=startsT[:, c, :], in_=pt[:, 0:BL])
                pt2 = ps2.tile([128, 16], F32, tag="ps2")
                nc.tensor.transpose(pt2[:, 0:BL],
                                    entf[:, 1 + c * 128:1 + (c + 1) * 128],
                                    ident[:BL, :BL])
                nc.vector.tensor_copy(out=entT[:, c, :], in_=pt2[:, 0:BL])

            spanT = spool.tile([128, TC, BL], F32)
            for mc in range(TC):
                pc = ps2.tile([128, 16], F32, tag="ps2")
                for kc in range(mc + 1):
                    nc.tensor.matmul(pc[:, 0:BL],
                                     triu if kc == mc else ones128,
                                     startsT[:, kc, :],
                                     start=(kc == 0), stop=(kc == mc))
                nc.vector.tensor_mul(spanT[:, mc, :], pc[:, 0:BL], entT[:, mc, :])

            masks = spool.tile([128, TC, BL, 2], F32)
            for c in range(TC):
                nc.vector.tensor_scalar(out=masks[:, c, :, 0], in0=spanT[:, c, :],
                                        scalar1=1.0, scalar2=None, op0=OP.is_equal)
                nc.vector.tensor_scalar(out=masks[:, c, :, 1], in0=spanT[:, c, :],
                                        scalar1=2.0, scalar2=None, op0=OP.is_equal)

            # zero-padded block-column masks: col 2b+j of masksZ[:,c,b,:] holds
            # mask j of example b, other columns zero, so each (b,c) matmul
            # accumulates only into rows 2b:2b+2 of the shared [16,768] psum.
            # zero background produced on device: memset cannot encode f32r,
            # but a DVE TensorCopy with f32r output can (f32 -> f32r rounding)
            zf32 = spool.tile([128, TC * BL * 2 * BL], F32)
            nc.vector.memset(zf32, 0.0)
            masksZ = spool.tile([128, TC, BL, 2 * BL], F32R)
            nc.vector.tensor_copy(
                out=masksZ.rearrange("p c b k -> p (c b k)"), in_=zf32)
            for c in range(TC):
                for b in range(BL):
                    nc.vector.tensor_copy(out=masksZ[:, c, b, 2 * b:2 * b + 2],
                                          in_=masks[:, c, b, :])

            # counts -> 1/max(c,1), rows 2b+j
            pcnt = ps1.tile([16, 1], F32, tag="cnt")
            for c in range(TC):
                nc.tensor.matmul(pcnt, masks[:, c], ones128[:, 0:1],
                                 start=(c == 0), stop=(c == TC - 1))
            cnt_sb = spool.tile([16, 1], F32)
            nc.vector.tensor_scalar_max(cnt_sb, pcnt, 1.0)
            recip = spool.tile([16, 1], F32)
            nc.vector.reciprocal(recip, cnt_sb)

            # pooled transposed early (used by the logits matmuls at the tail)
            pooledT = spool.tile([128, HC, BL], F32)
            for hc in range(HC):
                pp = ps2.tile([128, 16], F32, tag="ps2")
                nc.tensor.transpose(pp[:, 0:BL],
                                    pooled_sb[:, hc * 128:(hc + 1) * 128],
                                    ident[:BL, :BL])
                nc.vector.tensor_copy(out=pooledT[:, hc, :], in_=pp[:, 0:BL])

            # ---- weight DMAs (interleaved with h stream below) ----
            wcat_sb = wpool.tile([128, 3 * HC, L], F32)
            wT_sb = [wpool.tile([128, HC, H], F32, tag=f"wT{j}", name=f"wT{j}")
                     for j in range(3)]
            wT_d = [wTcls_d, wTe1_d, wTe2_d]

            # ---- h stream + masked segment-sum matmuls (fp32r) ----
            pe1 = pse.tile([16, 512], F32, tag="pe1")
            pe2 = pse.tile([16, 256], F32, tag="pe2")
            for b in range(BL):
                for c in range(TC):
                    if b == 0:
                        ht = ht0[c]
                    else:
                        ht = hpool.tile([128, H], F32R, tag="h")
                        nc.sync.dma_start(out=ht, in_=hv[:, b, c].bitcast(F32R))
                    lhsT = masksZ[:, c, b, :]
                    nc.tensor.matmul(pe1, lhsT, ht[:, 0:512],
                                     start=(b == 0 and c == 0),
                                     stop=(b == BL - 1 and c == TC - 1))
                    nc.tensor.matmul(pe2, lhsT, ht[:, 512:H],
                                     start=(b == 0 and c == 0),
                                     stop=(b == BL - 1 and c == TC - 1))
                # weight loads threaded between example streams: on-chip well
                # before the fold matmuls need them, without delaying h tiles
                if b == 1:
                    nc.sync.dma_start(out=wcat_sb, in_=wcat_d[:])
                if b in (2, 3, 4):
                    j = b - 2
                    nc.sync.dma_start(out=wT_sb[j], in_=wT_d[j][:])

            # ---- weight-chain folding on the otherwise idle TensorE ----
            # Wc_j[k, l] = sum_m W_j[k, m] Wcat_j[m, l]  (lhsT = W_j^T natural)
            wc_sb = [wpool.tile([128, HC, L], F32, tag=f"wc{j}", name=f"wc{j}")
                     for j in range(3)]
            for j in range(3):
                for kc in range(HC):
                    pf = ps2.tile([128, 32], F32, tag="ps2")
                    for mc in range(HC):
                        nc.tensor.matmul(pf[:, 0:L], wT_sb[j][:, mc, kc * 128:(kc + 1) * 128],
                                         wcat_sb[:, j * HC + mc, :],
                                         start=(mc == 0), stop=(mc == HC - 1))
                    nc.vector.tensor_copy(out=wc_sb[j][:, kc, :], in_=pf[:, 0:L])

            # bias_eff = b_cls@Wcat_0 + b_e1@Wcat_1 + b_e2@Wcat_2 + b_cat
            pb = ps1.tile([16, 32], F32, tag="cnt")
            for j in range(3):
                for mc in range(HC):
                    nc.tensor.matmul(pb[0:1, 0:L], b3_sb[:, mc, j:j + 1],
                                     wcat_sb[:, j * HC + mc, :],
                                     start=(j == 0 and mc == 0), stop=False)
            nc.tensor.matmul(pb[0:1, 0:L], ones128[0:1, 0:1], bcat_sb[0:1, :],
                             start=False, stop=True)
            beff_sb = spool.tile([1, L], F32)
            nc.vector.tensor_copy(out=beff_sb, in_=pb[0:1, 0:L])

            # ---- tail: tanh, transpose, 19 small matmuls, softmax, loss ----
            t12 = spool.tile([16, H], F32)
            t12T = spool.tile([128, HC, BL, 2], F32)
            nc.scalar.activation(out=t12[:, 0:512], in_=pe1, func=AF.Tanh,
                                 scale=recip)
            nc.scalar.activation(out=t12[:, 512:H], in_=pe2, func=AF.Tanh,
                                 scale=recip)
            for hc in range(HC):
                pt = ps2.tile([128, 16], F32, tag="ps2")
                nc.tensor.transpose(pt, t12[:, hc * 128:(hc + 1) * 128],
                                    ident[:16, :16])
                nc.vector.tensor_copy(out=t12T[:, hc], in_=pt)

            # logits = pooled@Wc0 + t1@Wc1 + t2@Wc2 + bias_eff
            plog = ps1.tile([BL, L], F32, tag="log")
            n_mm = 3 * HC
            i = 0
            for kc in range(HC):
                nc.tensor.matmul(plog, pooledT[:, kc, :], wc_sb[0][:, kc, :],
                                 start=(i == 0), stop=False)
                i += 1
            for j in (1, 2):
                for kc in range(HC):
                    nc.tensor.matmul(plog, t12T[:, kc, :, j - 1],
                                     wc_sb[j][:, kc, :],
                                     start=False, stop=False)
                    i += 1
            nc.tensor.matmul(plog, ones128[0:1, 0:BL], beff_sb[0:1, :],
                             start=False, stop=True)

            # probs = softmax(logits)
            out_sb = spool.tile([BL, L + 1], F32)
            nmx = spool.tile([BL, 1], F32)
            nc.vector.reduce_max(out=nmx, in_=plog, axis=mybir.AxisListType.X,
                                 negate=True)
            esb = spool.tile([BL, L], F32)
            ssb = spool.tile([BL, 1], F32)
            nc.scalar.activation(out=esb, in_=plog, func=AF.Exp, bias=nmx,
                                 accum_out=ssb)
            rs = spool.tile([BL, 1], F32)
            nc.vector.reciprocal(rs, ssb)
            nc.vector.tensor_scalar_mul(out_sb[:, 0:L], esb, rs)

            # per-example -loss contribution: log_softmax(probs)[label].
            # probs in [0,1] so exp(probs) needs no max-shift; exp(esb*rs)
            # fuses the normalize so it runs parallel with the probs mul.
            # ln computed on DVE via exponent extraction + atanh series
            # (avoids the 1.3us ACT natural_log table swap on the tail).
            e2sb = spool.tile([BL, L], F32)
            s2sb = spool.tile([BL, 1], F32)
            nc.scalar.activation(out=e2sb, in_=esb, func=AF.Exp, scale=rs,
                                 accum_out=s2sb)

            # ln(s2): s2 = m * 2^e, ln = e*ln2 + 2*atanh((m-1)/(m+1))
            bits = s2sb.bitcast(I32)
            e_i = spool.tile([BL, 1], I32)
            nc.vector.tensor_single_scalar(out=e_i, in_=bits, scalar=23,
                                           op=OP.arith_shift_right)
            e_f = spool.tile([BL, 1], F32)
            nc.vector.tensor_copy(out=e_f, in_=e_i)  # biased exponent e+127
            m_i = spool.tile([BL, 1], I32)
            nc.vector.tensor_scalar(out=m_i, in0=bits, scalar1=0x007FFFFF,
                                    scalar2=0x3F800000, op0=OP.bitwise_and,
                                    op1=OP.bitwise_or)
            m_f = m_i.bitcast(F32)               # mantissa in [1, 2)
            num = spool.tile([BL, 1], F32)
            nc.vector.tensor_scalar_sub(num, m_f, 1.0)
            den = spool.tile([BL, 1], F32)
            nc.vector.tensor_scalar_add(den, m_f, 1.0)
            rden = spool.tile([BL, 1], F32)
            nc.vector.reciprocal(rden, den)
            w = spool.tile([BL, 1], F32)
            nc.vector.tensor_mul(w, num, rden)   # w in [0, 1/3]
            w2 = spool.tile([BL, 1], F32)
            nc.vector.tensor_mul(w2, w, w)
            poly = spool.tile([BL, 1], F32)
            nc.vector.tensor_scalar(out=poly, in0=w2, scalar1=1.0 / 7.0,
                                    scalar2=1.0 / 5.0, op0=OP.mult, op1=OP.add)
            nc.vector.tensor_scalar(out=poly, in0=poly, scalar1=w2,
                                    scalar2=1.0 / 3.0, op0=OP.mult, op1=OP.add)
            nc.vector.tensor_scalar(out=poly, in0=poly, scalar1=w2,
                                    scalar2=1.0, op0=OP.mult, op1=OP.add)
            atanh = spool.tile([BL, 1], F32)
            nc.vector.tensor_mul(atanh, poly, w)  # atanh(w)
            eln2 = spool.tile([BL, 1], F32)
            nc.vector.tensor_scalar(out=eln2, in0=e_f,
                                    scalar1=float(np.log(2.0)),
                                    scalar2=float(127.0 * np.log(2.0)),
                                    op0=OP.mult, op1=OP.subtract)
            ln2t = spool.tile([BL, 1], F32)
            nc.vector.tensor_scalar(out=ln2t, in0=atanh, scalar1=2.0,
                                    scalar2=eln2, op0=OP.mult, op1=OP.add)

            labf = spool.tile([BL, 1], F32)
            nc.vector.tensor_copy(out=labf, in_=lab_sb)
            onehot = spool.tile([BL, L], F32)
            nc.vector.tensor_scalar(out=onehot, in0=iota30, scalar1=labf,
                                    scalar2=None, op0=OP.is_equal)
            pk30 = spool.tile([BL, L], F32)
            nc.vector.tensor_mul(pk30, out_sb[:, 0:L], onehot)
            pk = spool.tile([BL, 1], F32)
            nc.vector.reduce_sum(out=pk, in_=pk30, axis=mybir.AxisListType.X)
            nc.vector.tensor_sub(out_sb[:, L:L + 1], pk, ln2t)

            nc.sync.dma_start(out=out_d[:], in_=out_sb)

    nc.compile()
    return nc


def get_nc():
    if "nc" not in _CACHE:
        _CACHE["nc"] = build_nc()
    return _CACHE["nc"]


def make_in_maps(inputs):
    f32 = lambda x: np.ascontiguousarray(np.asarray(x), dtype=np.float32)
    i32 = lambda x: np.ascontiguousarray(np.asarray(x), dtype=np.int32)
    h = f32(inputs["last_hidden_states"])
    pooled = f32(inputs["pooled_output"])
    ent = i32(inputs["ent_ids"])
    labels = i32(inputs["labels"]).reshape(B, 1)
    def chunk_tile(mat):  # [R, C] -> [128, R//128, C] partition-contiguous
        r, c = mat.shape
        return np.ascontiguousarray(
            mat.reshape(r // 128, 128, c).transpose(1, 0, 2))

    wT_cls = chunk_tile(f32(np.asarray(inputs["W_cls"]).T))
    wT_e1 = chunk_tile(f32(np.asarray(inputs["W_e1"]).T))
    wT_e2 = chunk_tile(f32(np.asarray(inputs["W_e2"]).T))
    w_cat = chunk_tile(f32(inputs["W_cat"]))
    b3 = np.stack([f32(inputs["b_cls"]), f32(inputs["b_e1"]),
                   f32(inputs["b_e2"])], axis=1)
    b3 = np.ascontiguousarray(b3, dtype=np.float32)
    b_cat = f32(inputs["b_cat"]).reshape(1, L)

    in_maps = []
    for c in range(NCORES):
        sl = slice(c * BL, (c + 1) * BL)
        in_maps.append({
            "hid": h[sl], "pooled": pooled[sl], "ent": ent[sl],
            "labels": labels[sl],
            "wT_cls": wT_cls, "wT_e1": wT_e1, "wT_e2": wT_e2,
            "b3": b3, "w_cat": w_cat, "b_cat": b_cat,
        })
    return in_maps


def postprocess(outs):
    probs = np.concatenate([o[:, 0:L] for o in outs], axis=0).astype(np.float32)
    lossp = np.concatenate([o[:, L] for o in outs], axis=0)
    loss = np.asarray(-lossp.astype(np.float64).mean(), dtype=np.float32)
    return probs, loss


def kernel(**inputs):
    from concourse import bass_utils
    nc = get_nc()
    res = bass_utils.run_bass_kernel_spmd(nc, make_in_maps(inputs),
                                          core_ids=list(range(NCORES)))
    return postprocess([r["out"] for r in res.results])
